# revision 4
# baseline (speedup 1.0000x reference)
"""CenterLoss kernel for 8 TRN2 NeuronCores.

Math: with labels = argmax(y, 1), C' = codebook + scatter_add(sign(h)),
t = sign_with_random_zeros(C'[labels]):

    loss = alpha * (0.5*sum(h^2) + 0.5*B*BIT - sum_cj [sgn(C'_cj)*A_cj
                                                       + (C'_cj==0)*Z_cj])

where A = onehot^T @ h, Z = onehot^T @ (h*rs) are per-class sums and
Delta = onehot^T @ sign(h) is the (exactly integer) scatter-add delta.
This removes every gather/scatter: per 128-sample tile we build the
one-hot label matrix (row-max compare) and run accumulating matmuls.

Distribution: data-parallel over batch on 8 cores. Only Delta needs a
collective — it is AllReduced in two bf16 halves (the first hidden under
the second half of the main loop; bf16 is exact for these small integer
counts). A and Z stay core-local: each core dots its local A/Z against
the global sign/zero masks of C' and emits a partial loss; the host sums
the 8 partials. sum(h^2) rides along as a 9th matmul (diag of h^T h).
The codebook is passed pre-transposed ([bit, class]); alpha and the
0.5*B*BIT constant are applied on the host.
"""

import sys

if "/opt/trn_rl_repo" not in sys.path:
    sys.path.insert(0, "/opt/trn_rl_repo")

import numpy as np

B_FULL, BIT, N_CLASS, N_CORES = 65536, 128, 1000, 8
SUB = 128        # samples per tile (partition dim)
T_SUB = 4        # tiles per DMA super-tile

_compiled = {}


def build(b_shard):
    from concourse import bacc, mybir, tile

    f32 = mybir.dt.float32
    bf16 = mybir.dt.bfloat16
    i32 = mybir.dt.int32
    Alu = mybir.AluOpType
    AX = mybir.AxisListType

    n_tiles = b_shard // SUB
    n_super = b_shard // (SUB * T_SUB)
    assert n_super * SUB * T_SUB == b_shard
    assert n_tiles % 2 == 0
    half = n_tiles // 2  # Delta all-reduce split point

    nc = bacc.Bacc(
        "TRN2", target_bir_lowering=False, debug=False, num_devices=N_CORES
    )
    h = nc.dram_tensor("h", [b_shard, BIT], f32, kind="ExternalInput")
    y = nc.dram_tensor("y", [b_shard, N_CLASS], f32, kind="ExternalInput")
    cbT = nc.dram_tensor("cbT", [BIT, N_CLASS], f32, kind="ExternalInput")
    rs = nc.dram_tensor("rs", [b_shard, BIT], f32, kind="ExternalInput")
    out = nc.dram_tensor("out", [1, 1], f32, kind="ExternalOutput")

    with tile.TileContext(nc) as tc:
        with (
            tc.tile_pool(name="yio", bufs=3) as y_pool,
            tc.tile_pool(name="hio", bufs=3) as h_pool,
            tc.tile_pool(name="work", bufs=4) as work_pool,
            tc.tile_pool(name="acc", bufs=1) as acc_pool,
            tc.tile_pool(name="psum", bufs=1, space="PSUM") as psum_pool,
            tc.tile_pool(name="dram", bufs=1, space="DRAM") as dram_pool,
        ):
            psum_d = psum_pool.tile([SUB, N_CLASS], f32)   # 2 banks
            psum_a = psum_pool.tile([SUB, N_CLASS], f32)   # 2 banks
            psum_z = psum_pool.tile([SUB, N_CLASS], f32)   # 2 banks
            psum_q = psum_pool.tile([SUB, BIT], f32)       # 1 bank (h^T h)
            psum_s = psum_pool.tile([1, 1], f32)           # 1 bank

            cbT_sb = acc_pool.tile([BIT, N_CLASS], f32)
            nc.sync.dma_start(cbT_sb[:], cbT.ap()[:])

            # identity mask for extracting diag(h^T h)
            iota_t = acc_pool.tile([SUB, BIT], i32)
            nc.gpsimd.iota(iota_t[:], pattern=[[1, BIT]], base=0,
                           channel_multiplier=-1)
            ident = acc_pool.tile([SUB, BIT], bf16)
            nc.gpsimd.tensor_scalar(ident[:], iota_t[:], 0, None,
                                    op0=Alu.is_equal)

            stage1 = acc_pool.tile([SUB, N_CLASS], bf16)
            stage2 = acc_pool.tile([SUB, N_CLASS], bf16)
            cc1_in = dram_pool.tile([SUB, N_CLASS], bf16)
            cc1_out = dram_pool.tile([SUB, N_CLASS], bf16, addr_space="Shared")
            cc2_in = dram_pool.tile([SUB, N_CLASS], bf16)
            cc2_out = dram_pool.tile([SUB, N_CLASS], bf16, addr_space="Shared")

            y_re = y.ap().rearrange("(s t p) c -> s p t c", p=SUB, t=T_SUB)
            h_re = h.ap().rearrange("(s t p) c -> s p t c", p=SUB, t=T_SUB)
            rs_re = rs.ap().rearrange("(s t p) c -> s p t c", p=SUB, t=T_SUB)

            it = 0
            for s in range(n_super):
                y_sb = y_pool.tile([SUB, T_SUB, N_CLASS], f32, name="y_sb")
                h_sb = h_pool.tile([SUB, T_SUB, BIT], f32, name="h_sb")
                rs_sb = h_pool.tile([SUB, T_SUB, BIT], f32, name="rs_sb")
                nc.sync.dma_start(y_sb[:], y_re[s])
                nc.sync.dma_start(h_sb[:], h_re[s])
                nc.sync.dma_start(rs_sb[:], rs_re[s])
                for t in range(T_SUB):
                    y_t = y_sb[:, t, :]
                    h_t = h_sb[:, t, :]
                    rs_t = rs_sb[:, t, :]
                    first = it == 0
                    last = it == n_tiles - 1

                    rmax = work_pool.tile([SUB, 1], f32, name="rmax")
                    nc.vector.tensor_reduce(rmax[:], y_t, axis=AX.X, op=Alu.max)
                    onehot = work_pool.tile([SUB, N_CLASS], bf16, name="onehot")
                    nc.gpsimd.tensor_scalar(
                        onehot[:], y_t, rmax[:], None, op0=Alu.is_equal
                    )

                    sH = work_pool.tile([SUB, BIT], bf16, name="sH")
                    nc.scalar.sign(sH[:], h_t)
                    hbf = work_pool.tile([SUB, BIT], bf16, name="hbf")
                    nc.scalar.copy(hbf[:], h_t)
                    hrs = work_pool.tile([SUB, BIT], bf16, name="hrs")
                    nc.vector.tensor_tensor(hrs[:], h_t, rs_t, op=Alu.mult)

                    # Delta accumulates in two groups split at `half` so the
                    # first AllReduce hides under the loop's second half.
                    d_first = it == 0 or it == half
                    d_last = it == half - 1 or it == n_tiles - 1
                    nc.tensor.matmul(psum_d[:, 0:512], sH[:], onehot[:, 0:512],
                                     start=d_first, stop=d_last)
                    nc.tensor.matmul(psum_d[:, 512:N_CLASS], sH[:],
                                     onehot[:, 512:N_CLASS],
                                     start=d_first, stop=d_last)
                    nc.tensor.matmul(psum_a[:, 0:512], hbf[:], onehot[:, 0:512],
                                     start=first, stop=last)
                    nc.tensor.matmul(psum_a[:, 512:N_CLASS], hbf[:],
                                     onehot[:, 512:N_CLASS],
                                     start=first, stop=last)
                    nc.tensor.matmul(psum_q[:], hbf[:], hbf[:],
                                     start=first, stop=last)
                    nc.tensor.matmul(psum_z[:, 0:512], hrs[:], onehot[:, 0:512],
                                     start=first, stop=last)
                    nc.tensor.matmul(psum_z[:, 512:N_CLASS], hrs[:],
                                     onehot[:, 512:N_CLASS],
                                     start=first, stop=last)
                    it += 1

                    if it == half:
                        # first-half Delta -> bf16 (exact: small ints) -> AR
                        nc.vector.tensor_copy(stage1[:], psum_d[:])
                        nc.sync.dma_start(cc1_in[:], stage1[:])
                        nc.gpsimd.collective_compute(
                            "AllReduce", Alu.add,
                            replica_groups=[list(range(N_CORES))],
                            ins=[cc1_in.opt()], outs=[cc1_out.opt()],
                        )

            # ---- tail ----
            nc.vector.tensor_copy(stage2[:], psum_d[:])
            nc.sync.dma_start(cc2_in[:], stage2[:])
            nc.gpsimd.collective_compute(
                "AllReduce", Alu.add,
                replica_groups=[list(range(N_CORES))],
                ins=[cc2_in.opt()], outs=[cc2_out.opt()],
            )
            ar1 = acc_pool.tile([SUB, N_CLASS], bf16)
            ar2 = acc_pool.tile([SUB, N_CLASS], bf16)
            nc.sync.dma_start(ar1[:], cc1_out[:])
            nc.sync.dma_start(ar2[:], cc2_out[:])

            # C' = cbT + Delta_total  (exact f32 integer arithmetic)
            cpr = acc_pool.tile([SUB, N_CLASS], f32)
            nc.vector.tensor_tensor(cpr[:], ar1[:], ar2[:], op=Alu.add)
            nc.vector.tensor_tensor(cpr[:], cpr[:], cbT_sb[:], op=Alu.add)

            # local dots of the global masks against this core's A and Z
            trash2 = acc_pool.tile([SUB, N_CLASS], f32)
            fpos = acc_pool.tile([SUB, 1], f32)
            fneg = acc_pool.tile([SUB, 1], f32)
            fzer = acc_pool.tile([SUB, 1], f32)
            nc.vector.scalar_tensor_tensor(
                trash2[:], cpr[:], 0.0, psum_a[:],
                op0=Alu.is_gt, op1=Alu.mult, accum_out=fpos[:],
            )
            nc.vector.scalar_tensor_tensor(
                trash2[:], cpr[:], 0.0, psum_a[:],
                op0=Alu.is_lt, op1=Alu.mult, accum_out=fneg[:],
            )
            nc.vector.scalar_tensor_tensor(
                trash2[:], cpr[:], 0.0, psum_z[:],
                op0=Alu.is_equal, op1=Alu.mult, accum_out=fzer[:],
            )
            # qdiag[p] = (h^T h)[p, p] -> local sum(h^2) per bit row
            trashq = acc_pool.tile([SUB, BIT], f32)
            qdiag = acc_pool.tile([SUB, 1], f32)
            nc.vector.scalar_tensor_tensor(
                trashq[:], psum_q[:], 1.0, ident[:],
                op0=Alu.mult, op1=Alu.mult, accum_out=qdiag[:],
            )

            # colv = 0.5*qdiag - fpos + fneg - fzer ; partial = sum_p colv
            colv = acc_pool.tile([SUB, 1], f32)
            nc.vector.tensor_scalar(colv[:], qdiag[:], 0.5, None, op0=Alu.mult)
            nc.vector.tensor_tensor(colv[:], colv[:], fpos[:], op=Alu.subtract)
            nc.vector.tensor_tensor(colv[:], colv[:], fneg[:], op=Alu.add)
            nc.vector.tensor_tensor(colv[:], colv[:], fzer[:], op=Alu.subtract)

            ones = acc_pool.tile([SUB, 1], f32)
            nc.vector.memset(ones[:], 1.0)
            nc.tensor.matmul(psum_s[:], colv[:], ones[:], start=True, stop=True)
            out_sb = acc_pool.tile([1, 1], f32)
            nc.vector.tensor_copy(out_sb[:], psum_s[:])
            nc.sync.dma_start(out.ap()[:], out_sb[:])

    nc.compile()
    return nc


def _get_compiled(b_shard):
    nc = _compiled.get(b_shard)
    if nc is None:
        nc = build(b_shard)
        _compiled[b_shard] = nc
    return nc


def make_in_maps(h, y, cb, rs):
    b_shard = h.shape[0] // N_CORES
    cbT = np.ascontiguousarray(cb.T, dtype=np.float32)
    in_maps = []
    for i in range(N_CORES):
        sl = slice(i * b_shard, (i + 1) * b_shard)
        in_maps.append(
            {
                "h": np.ascontiguousarray(h[sl], dtype=np.float32),
                "y": np.ascontiguousarray(y[sl], dtype=np.float32),
                "cbT": cbT,
                "rs": np.ascontiguousarray(rs[sl], dtype=np.float32),
            }
        )
    return in_maps


def finish(results, b_full, alpha):
    partials = sum(float(r["out"][0, 0]) for r in results)
    return np.float32((partials + 0.5 * b_full * BIT) * float(alpha))


def run(inputs, trace=False, trace_kwargs=None):
    """Run on hardware; returns (loss_scalar_f32, BassKernelResults)."""
    from concourse import bass_utils

    h = inputs["h"]
    b_shard = h.shape[0] // N_CORES
    nc = _get_compiled(b_shard)
    in_maps = make_in_maps(h, inputs["y"], inputs["codebook"], inputs["rand_signs"])
    res = bass_utils.run_bass_kernel_spmd(
        nc,
        in_maps,
        core_ids=list(range(N_CORES)),
        trace=trace,
        **(trace_kwargs or {}),
    )
    alpha = float(np.asarray(inputs.get("alpha", 1)))
    return finish(res.results, h.shape[0], alpha), res


def kernel(**inputs) -> np.ndarray:
    loss, _ = run(inputs)
    return loss


# revision 6
# speedup vs baseline: 5.1992x; 5.1992x over previous
"""CenterLoss kernel for 8 TRN2 NeuronCores.

Math: with labels = argmax(y, 1), C' = codebook + scatter_add(sign(h)),
t = sign_with_random_zeros(C'[labels]):

    loss = alpha * (0.5*sum(h^2) + 0.5*B*BIT - sum_cj [sgn(C'_cj)*A_cj
                                                       + (C'_cj==0)*Z_cj])

where A = onehot^T @ h, Z = onehot^T @ (h*rs) are per-class sums and
Delta = onehot^T @ sign(h) is the (exactly integer) scatter-add delta.
This removes every gather/scatter: per 128-sample tile we build the
one-hot label matrix (row-max compare) and run accumulating matmuls.

Distribution: data-parallel over batch on 8 cores. Only Delta needs a
collective — it is AllReduced in two bf16 halves (the first hidden under
the second half of the main loop; bf16 is exact for these small integer
counts). A and Z stay core-local: each core dots its local A/Z against
the global sign/zero masks of C' and emits a partial loss; the host sums
the 8 partials. sum(h^2) rides along as a 9th matmul (diag of h^T h).
The codebook is passed pre-transposed ([bit, class]); alpha and the
0.5*B*BIT constant are applied on the host.
"""

import sys

if "/opt/trn_rl_repo" not in sys.path:
    sys.path.insert(0, "/opt/trn_rl_repo")

import numpy as np

B_FULL, BIT, N_CLASS, N_CORES = 65536, 128, 1000, 8
SUB = 128        # samples per tile (partition dim)
T_SUB = 4        # tiles per DMA super-tile

_compiled = {}


def build(b_shard):
    from concourse import bacc, mybir, tile

    f32 = mybir.dt.float32
    bf16 = mybir.dt.bfloat16
    i32 = mybir.dt.int32
    Alu = mybir.AluOpType
    AX = mybir.AxisListType

    n_tiles = b_shard // SUB
    n_super = b_shard // (SUB * T_SUB)
    assert n_super * SUB * T_SUB == b_shard
    assert n_tiles % 2 == 0
    half = n_tiles // 2  # Delta all-reduce split point

    nc = bacc.Bacc(
        "TRN2", target_bir_lowering=False, debug=False, num_devices=N_CORES
    )
    h = nc.dram_tensor("h", [b_shard, BIT], f32, kind="ExternalInput")
    y = nc.dram_tensor("y", [b_shard, N_CLASS], f32, kind="ExternalInput")
    cbT = nc.dram_tensor("cbT", [BIT, N_CLASS], f32, kind="ExternalInput")
    rs = nc.dram_tensor("rs", [b_shard, BIT], f32, kind="ExternalInput")
    out = nc.dram_tensor("out", [1, 1], f32, kind="ExternalOutput")

    with tile.TileContext(nc) as tc:
        with (
            tc.tile_pool(name="yio", bufs=3) as y_pool,
            tc.tile_pool(name="hio", bufs=3) as h_pool,
            tc.tile_pool(name="work", bufs=4) as work_pool,
            tc.tile_pool(name="acc", bufs=1) as acc_pool,
            tc.tile_pool(name="psum", bufs=1, space="PSUM") as psum_pool,
            tc.tile_pool(name="dram", bufs=1, space="DRAM") as dram_pool,
        ):
            psum_d = psum_pool.tile([SUB, N_CLASS], f32)   # 2 banks
            psum_a = psum_pool.tile([SUB, N_CLASS], f32)   # 2 banks
            psum_z = psum_pool.tile([SUB, N_CLASS], f32)   # 2 banks
            psum_q = psum_pool.tile([SUB, BIT], f32)       # 1 bank (h^T h)
            psum_s = psum_pool.tile([1, 1], f32)           # 1 bank

            cbT_sb = acc_pool.tile([BIT, N_CLASS], f32)
            nc.sync.dma_start(cbT_sb[:], cbT.ap()[:])

            # identity mask for extracting diag(h^T h)
            iota_t = acc_pool.tile([SUB, BIT], i32)
            nc.gpsimd.iota(iota_t[:], pattern=[[1, BIT]], base=0,
                           channel_multiplier=-1)
            ident = acc_pool.tile([SUB, BIT], bf16)
            nc.gpsimd.tensor_scalar(ident[:], iota_t[:], 0, None,
                                    op0=Alu.is_equal)

            stage1 = acc_pool.tile([SUB, N_CLASS], bf16)
            stage2 = acc_pool.tile([SUB, N_CLASS], bf16)
            cc1_in = dram_pool.tile([SUB, N_CLASS], bf16)
            cc1_out = dram_pool.tile([SUB, N_CLASS], bf16, addr_space="Shared")
            cc2_in = dram_pool.tile([SUB, N_CLASS], bf16)
            cc2_out = dram_pool.tile([SUB, N_CLASS], bf16, addr_space="Shared")

            # partition p holds T_SUB consecutive batch rows -> one large
            # contiguous DMA descriptor per partition per super-tile
            y_re = y.ap().rearrange("(s p t) c -> s p t c", p=SUB, t=T_SUB)
            h_re = h.ap().rearrange("(s p t) c -> s p t c", p=SUB, t=T_SUB)
            rs_re = rs.ap().rearrange("(s p t) c -> s p t c", p=SUB, t=T_SUB)

            it = 0
            for s in range(n_super):
                y_sb = y_pool.tile([SUB, T_SUB, N_CLASS], f32, name="y_sb")
                h_sb = h_pool.tile([SUB, T_SUB, BIT], f32, name="h_sb")
                rs_sb = h_pool.tile([SUB, T_SUB, BIT], f32, name="rs_sb")
                nc.sync.dma_start(y_sb[:], y_re[s])
                nc.sync.dma_start(h_sb[:], h_re[s])
                nc.sync.dma_start(rs_sb[:], rs_re[s])
                for t in range(T_SUB):
                    y_t = y_sb[:, t, :]
                    h_t = h_sb[:, t, :]
                    rs_t = rs_sb[:, t, :]
                    first = it == 0
                    last = it == n_tiles - 1

                    rmax = work_pool.tile([SUB, 1], f32, name="rmax")
                    nc.vector.tensor_reduce(rmax[:], y_t, axis=AX.X, op=Alu.max)
                    onehot = work_pool.tile([SUB, N_CLASS], bf16, name="onehot")
                    nc.vector.tensor_scalar(
                        onehot[:], y_t, rmax[:], None, op0=Alu.is_equal
                    )

                    sH = work_pool.tile([SUB, BIT], bf16, name="sH")
                    nc.scalar.sign(sH[:], h_t)
                    hbf = work_pool.tile([SUB, BIT], bf16, name="hbf")
                    nc.scalar.copy(hbf[:], h_t)
                    hrs = work_pool.tile([SUB, BIT], bf16, name="hrs")
                    nc.vector.tensor_tensor(hrs[:], h_t, rs_t, op=Alu.mult)

                    # Delta accumulates in two groups split at `half` so the
                    # first AllReduce hides under the loop's second half.
                    d_first = it == 0 or it == half
                    d_last = it == half - 1 or it == n_tiles - 1
                    nc.tensor.matmul(psum_d[:, 0:512], sH[:], onehot[:, 0:512],
                                     start=d_first, stop=d_last)
                    nc.tensor.matmul(psum_d[:, 512:N_CLASS], sH[:],
                                     onehot[:, 512:N_CLASS],
                                     start=d_first, stop=d_last)
                    nc.tensor.matmul(psum_a[:, 0:512], hbf[:], onehot[:, 0:512],
                                     start=first, stop=last)
                    nc.tensor.matmul(psum_a[:, 512:N_CLASS], hbf[:],
                                     onehot[:, 512:N_CLASS],
                                     start=first, stop=last)
                    nc.tensor.matmul(psum_q[:], hbf[:], hbf[:],
                                     start=first, stop=last)
                    nc.tensor.matmul(psum_z[:, 0:512], hrs[:], onehot[:, 0:512],
                                     start=first, stop=last)
                    nc.tensor.matmul(psum_z[:, 512:N_CLASS], hrs[:],
                                     onehot[:, 512:N_CLASS],
                                     start=first, stop=last)
                    it += 1

                    if it == half:
                        # first-half Delta -> bf16 (exact: small ints) -> AR
                        nc.vector.tensor_copy(stage1[:], psum_d[:])
                        nc.sync.dma_start(cc1_in[:], stage1[:])
                        nc.gpsimd.collective_compute(
                            "AllReduce", Alu.add,
                            replica_groups=[list(range(N_CORES))],
                            ins=[cc1_in.opt()], outs=[cc1_out.opt()],
                        )

            # ---- tail ----
            nc.vector.tensor_copy(stage2[:], psum_d[:])
            nc.sync.dma_start(cc2_in[:], stage2[:])
            nc.gpsimd.collective_compute(
                "AllReduce", Alu.add,
                replica_groups=[list(range(N_CORES))],
                ins=[cc2_in.opt()], outs=[cc2_out.opt()],
            )
            ar1 = acc_pool.tile([SUB, N_CLASS], bf16)
            ar2 = acc_pool.tile([SUB, N_CLASS], bf16)
            nc.sync.dma_start(ar1[:], cc1_out[:])
            nc.sync.dma_start(ar2[:], cc2_out[:])

            # C' = cbT + Delta_total  (exact f32 integer arithmetic)
            cpr = acc_pool.tile([SUB, N_CLASS], f32)
            nc.vector.tensor_tensor(cpr[:], ar1[:], ar2[:], op=Alu.add)
            nc.vector.tensor_tensor(cpr[:], cpr[:], cbT_sb[:], op=Alu.add)

            # local dots of the global masks against this core's A and Z
            trash2 = acc_pool.tile([SUB, N_CLASS], f32)
            fpos = acc_pool.tile([SUB, 1], f32)
            fneg = acc_pool.tile([SUB, 1], f32)
            fzer = acc_pool.tile([SUB, 1], f32)
            nc.vector.scalar_tensor_tensor(
                trash2[:], cpr[:], 0.0, psum_a[:],
                op0=Alu.is_gt, op1=Alu.mult, accum_out=fpos[:],
            )
            nc.vector.scalar_tensor_tensor(
                trash2[:], cpr[:], 0.0, psum_a[:],
                op0=Alu.is_lt, op1=Alu.mult, accum_out=fneg[:],
            )
            nc.vector.scalar_tensor_tensor(
                trash2[:], cpr[:], 0.0, psum_z[:],
                op0=Alu.is_equal, op1=Alu.mult, accum_out=fzer[:],
            )
            # qdiag[p] = (h^T h)[p, p] -> local sum(h^2) per bit row
            trashq = acc_pool.tile([SUB, BIT], f32)
            qdiag = acc_pool.tile([SUB, 1], f32)
            nc.vector.scalar_tensor_tensor(
                trashq[:], psum_q[:], 1.0, ident[:],
                op0=Alu.mult, op1=Alu.mult, accum_out=qdiag[:],
            )

            # colv = 0.5*qdiag - fpos + fneg - fzer ; partial = sum_p colv
            colv = acc_pool.tile([SUB, 1], f32)
            nc.vector.tensor_scalar(colv[:], qdiag[:], 0.5, None, op0=Alu.mult)
            nc.vector.tensor_tensor(colv[:], colv[:], fpos[:], op=Alu.subtract)
            nc.vector.tensor_tensor(colv[:], colv[:], fneg[:], op=Alu.add)
            nc.vector.tensor_tensor(colv[:], colv[:], fzer[:], op=Alu.subtract)

            ones = acc_pool.tile([SUB, 1], f32)
            nc.vector.memset(ones[:], 1.0)
            nc.tensor.matmul(psum_s[:], colv[:], ones[:], start=True, stop=True)
            out_sb = acc_pool.tile([1, 1], f32)
            nc.vector.tensor_copy(out_sb[:], psum_s[:])
            nc.sync.dma_start(out.ap()[:], out_sb[:])

    nc.compile()
    return nc


def _get_compiled(b_shard):
    nc = _compiled.get(b_shard)
    if nc is None:
        nc = build(b_shard)
        _compiled[b_shard] = nc
    return nc


def make_in_maps(h, y, cb, rs):
    b_shard = h.shape[0] // N_CORES
    cbT = np.ascontiguousarray(cb.T, dtype=np.float32)
    in_maps = []
    for i in range(N_CORES):
        sl = slice(i * b_shard, (i + 1) * b_shard)
        in_maps.append(
            {
                "h": np.ascontiguousarray(h[sl], dtype=np.float32),
                "y": np.ascontiguousarray(y[sl], dtype=np.float32),
                "cbT": cbT,
                "rs": np.ascontiguousarray(rs[sl], dtype=np.float32),
            }
        )
    return in_maps


def finish(results, b_full, alpha):
    partials = sum(float(r["out"][0, 0]) for r in results)
    return np.float32((partials + 0.5 * b_full * BIT) * float(alpha))


def run(inputs, trace=False, trace_kwargs=None):
    """Run on hardware; returns (loss_scalar_f32, BassKernelResults)."""
    from concourse import bass_utils

    h = inputs["h"]
    b_shard = h.shape[0] // N_CORES
    nc = _get_compiled(b_shard)
    in_maps = make_in_maps(h, inputs["y"], inputs["codebook"], inputs["rand_signs"])
    res = bass_utils.run_bass_kernel_spmd(
        nc,
        in_maps,
        core_ids=list(range(N_CORES)),
        trace=trace,
        **(trace_kwargs or {}),
    )
    alpha = float(np.asarray(inputs.get("alpha", 1)))
    return finish(res.results, h.shape[0], alpha), res


def kernel(**inputs) -> np.ndarray:
    loss, _ = run(inputs)
    return loss
